# revision 42
# baseline (speedup 1.0000x reference)
"""Trainium2 Bass kernel for the CRF forward algorithm (nn_CRF).

Reference computes: scan over S=8192 steps of
    fv'[i] = logsumexp_j(fv[j] + transitions[i, j]) + h[s, i]
then logsumexp(fv + transitions[END_IDX]).

Algorithm (two levels):

1. Exp-space LINEAR scan with lagged normalizer.  W = exp(tr - ln2)
   (bf16, on device; the /2 scale is undone exactly on the host with
   +8192*ln2).  State is carried LINEARLY as w = exp(fv - C):
    per step s:
        mv      = W @ w_s                      (PE matvec, fp32 psum)
        mh_s    = ln(colsum . w_s)             (PE skinny matmul + ACT Ln)
        es_s    = h[s] - mh_{s-1}              (DVE, off critical path)
        t_es    = exp(es_s)                    (ACT, off critical path)
        w_{s+1} = mv * t_es                    (DVE elementwise, bf16)
        C      += mh_{s-1}
   Identity: exp(fv_s) = w_{s+1} * e^{C_s}, so the log-state is never
   needed on device -- the host takes ln() of the dumped w.  The lagged
   normalizer keeps w's dynamic range bounded (any mh sequence keeps
   fv = ln(w)+C exact); measured ln(w) stays in [-95, +13] so bf16
   never overflows and flushed tiny components are irrelevant.
   The ONLY serial cross-engine dependency per step is
   PE matvec -> DVE multiply -> PE next matvec; the mh/es/t_es chain
   runs concurrently with the matvec.  The w-multiply + 2 semaphore
   hops overlap the 16 skinny matmuls (pe_mv fires before them, w is
   double-buffered by step parity).

2. Sequence parallelism via filter forgetting: the CRF forward filter
   forgets its initial condition in <16 steps (measured: log-direction
   error ~1e-15 after 16 steps -- dense random transitions mix fast).
   Split S=8192 into 8 chunks with boundaries e_c = W0 + K*(c+1),
   K=(S-W0)/8.  Core 0 runs rows [0, e_0) from the true init (exact).
   Core c>=1 runs rows [e_{c-1}-W0, e_c) from a uniform init: after the
   W0-step warmup its state direction equals the true filter state at
   e_{c-1}; a snapshot (w_snap, C_snap) is taken there.  Chunk log-gain
   G_c = (C_fin + lse(ln w_fin)) - (C_snap + lse(ln w_snap)) is exact
   given the snapshot direction.  Host stitches in float64:
   ans = (C_fin_0 + lse_0) + sum_c G_c
         + lse(lnw_7 - lse_7 + transitions[END_IDX]) + 8192*ln2.
   Zero inter-core communication; each core does W0+K steps instead of
   8192.

Layout: tag j -> (partition p = j//16, slot k = j%16); w/h tiles are
[128, 16].  W^T lives in SBUF as 256 bf16 tiles [128 j, 128 i], tile
t = k*16 + g at free offset t*128 (k = j-slot, g = i-group).

(Cross-core remote-DMA crashes this runtime and in-loop collectives have
a ~5us floor -- both dead ends; the warmup scheme needs neither.)
"""
import sys

sys.path.insert(0, "/opt/trn_rl_repo")

import numpy as np

S = 8192
T = 2048
P = 128
NSLOT = T // P          # 16 j-slots
NGRP = T // P           # 16 i-groups
NBLK = NSLOT * NGRP     # 256 W tiles
UNROLL = 2              # steps per loop iteration (buffer parity)
LN2 = 0.6931471805599453
W0 = 16                 # warmup steps (forgetting window)
K = (S - W0) // 8       # chunk length = 1022
NSTEP = W0 + K          # per-core steps = 1038
NCORE = 8


def build_kernel(n_steps=NSTEP, snap_step=W0, timing_mode=False,
                 pe_only=False, wt_fp8=False):
    import concourse.bacc as bacc
    import concourse.bass as bass
    import concourse.mybir as mybir
    from contextlib import ExitStack

    assert n_steps % UNROLL == 0 and snap_step % UNROLL == 0
    assert 0 < snap_step < n_steps
    assert not pe_only or timing_mode
    fp32 = mybir.dt.float32
    bf16 = mybir.dt.bfloat16
    fp8 = mybir.dt.float8e4
    AF = mybir.ActivationFunctionType
    ALU = mybir.AluOpType
    AX = mybir.AxisListType

    nc = bacc.Bacc("TRN2", target_bir_lowering=True, num_devices=8)

    n_wtb = 2 if timing_mode else NBLK
    n_hsb = 2 if timing_mode else n_steps
    wtb = nc.declare_dram_parameter("wtb", [n_wtb, P, P], fp32, isOutput=False)
    hsb = nc.declare_dram_parameter("hsb", [n_hsb, T], fp32, isOutput=False)
    v0f = nc.declare_dram_parameter("v0f", [P, NSLOT], fp32, isOutput=False)
    wfin_d = nc.declare_dram_parameter("wfin", [P, NSLOT], fp32, isOutput=True)
    cfin_d = nc.declare_dram_parameter("cfin", [1, 1], fp32, isOutput=True)
    wsnp_d = nc.declare_dram_parameter("wsnp", [P, NSLOT], fp32, isOutput=True)
    csnp_d = nc.declare_dram_parameter("csnp", [1, 1], fp32, isOutput=True)

    ctx = ExitStack()
    sb = lambda name, shape, dt: ctx.enter_context(nc.sbuf_tensor(name, shape, dt))
    ps = lambda name, shape, dt: ctx.enter_context(nc.psum_tensor(name, shape, dt))
    sem = lambda name: ctx.enter_context(nc.semaphore(name))

    with ctx:
        wt = sb("wt", [P, NBLK * P], fp8 if wt_fp8 else bf16)
        colsum = sb("colsum", [P, NSLOT], fp32)
        colsum_bf = sb("colsum_bf", [P, NSLOT], bf16)
        v = sb("v", [P, NSLOT], fp32)        # init log-state (v0 load)
        wbuf = [sb(f"wbuf{i}", [P, NSLOT], bf16) for i in range(2)]
        tes = [sb(f"tes{i}", [P, NSLOT], fp32) for i in range(2)]
        esb = [sb(f"esb{i}", [P, NSLOT], fp32) for i in range(2)]
        h_step = [sb(f"h_step{i}", [P, NSLOT], fp32) for i in range(UNROLL)]
        tmp = [sb(f"tmp{i}", [P, P], fp32) for i in range(2)]
        ln2n_t = sb("ln2n_t", [P, 1], fp32)
        ones_row = sb("ones_row", [1, P], fp32)
        m_sb = sb("m_sb", [1, 1], fp32)      # mh scalar (lagged)
        c_acc = sb("c_acc", [1, 1], fp32)    # C accumulator
        w_snap = sb("w_snap", [P, NSLOT], fp32)
        c_snap = sb("c_snap", [1, 1], fp32)
        w_fin = sb("w_fin", [P, NSLOT], fp32)

        psum_mv = ps("psum_mv", [P, NSLOT], fp32)
        psum_m = ps("psum_m", [1, 1], fp32)
        psum_b = ps("psum_b", [P, 1], fp32)

        su_dma = [sem("su_dma0"), sem("su_dma1")]  # wtb DMAs by parity
        su_exp = sem("su_exp")       # setup exp done (+1 per block)
        su_misc = sem("su_misc")     # consts / v0 ready
        h_ready = [sem("h_ready0"), sem("h_ready1")]  # h DMA by parity
        w_sem = sem("w_sem")         # w(s) ready (init exp; then DVE mult)
        pe_mv = sem("pe_mv")         # PE matvec done (+1 per step)
        pe1 = sem("pe1")             # PE matvec+skinny done (+1 per step)
        pe2 = sem("pe2")             # PE mh-bcast done (+1 per step)
        act_tes = sem("act_tes")     # ACT exp(es) done (+1 per step)
        act_m = sem("act_m")         # ACT Ln(psum_m)->m_sb done (+1 per step)
        es_sem = sem("es_sem")       # DVE es + c_acc done (+1 per step)
        snap_sem = sem("snap_sem")   # snapshot copies done
        fin_sem = sem("fin_sem")     # final copies done
        gp_done = sem("gp_done")     # gpsimd consts ready

        n_iter = n_steps // UNROLL
        snap_iter = snap_step // UNROLL

        with nc.Block() as block:

            # ---------------- sync engine: all input DMAs ----------------
            @block.sync
            def _(eng):
                eng.dma_start(v[:, :], v0f[:, :]).then_inc(su_misc, 16)
                for t in range(NBLK):
                    if t >= 2:
                        eng.wait_ge(su_exp, t - 1)
                    eng.dma_start(
                        tmp[t % 2][:, :],
                        wtb[(t % 2 if timing_mode else t), :, :],
                    ).then_inc(su_dma[t % 2], 16)
                # h prologue: steps 0..UNROLL-1
                for s in range(UNROLL):
                    eng.dma_start(
                        h_step[s][:, :], hsb[s : s + 1, :]
                    ).then_inc(h_ready[s % 2], 16)
                r_off = eng.alloc_register("r_off")   # step index
                r_g = eng.alloc_register("r_g")       # pacing guard
                r_i = eng.alloc_register("r_i")
                eng.reg_mov(r_off, UNROLL)
                eng.reg_mov(r_g, 0)
                eng.reg_mov(r_i, 0)
                gate = pe1 if pe_only else es_sem
                eng.br("sync_loop")
                with nc.body("sync_loop"):
                    for u in range(UNROLL):
                        eng.reg_add(r_g, r_g, 1)
                        eng.wait_ge(gate, r_g)
                        eng.dma_start(
                            h_step[u][:, :],
                            hsb[u : u + 1, :]
                            if timing_mode
                            else hsb[bass.ds(eng.snap(r_off), 1), :],
                        ).then_inc(h_ready[u % 2], 16)
                        if not timing_mode:
                            eng.reg_add(r_off, r_off, 1)
                    eng.reg_add(r_i, r_i, 1)
                    eng.br_lt(r_i, n_iter - 1, "sync_loop", "sync_done")
                with nc.body("sync_done"):
                    if pe_only:
                        eng.wait_ge(pe1, n_steps)
                    else:
                        eng.wait_ge(snap_sem, 1)
                        eng.wait_ge(fin_sem, 1)
                    eng.dma_start(wfin_d[:, :], w_fin[:, :]).then_inc(su_misc, 16)
                    eng.dma_start(cfin_d[:, :], c_acc[:, :]).then_inc(su_misc, 16)
                    eng.dma_start(wsnp_d[:, :], w_snap[:, :]).then_inc(su_misc, 16)
                    eng.dma_start(csnp_d[:, :], c_snap[:, :]).then_inc(su_misc, 16)
                    eng.wait_ge(su_misc, 112)
                    eng.br(block.end_bb)

            # ---------------- gpsimd: constants only ----------------
            @block.gpsimd
            def _(eng):
                eng.memset(ln2n_t[:, :], -LN2)
                eng.memset(ones_row[:, :], 1.0)
                eng.memset(c_acc[:, :], 0.0)
                eng.memset(m_sb[:, :], 0.0)
                eng.drain()
                eng.nop().then_inc(su_misc, 16)
                eng.nop().then_inc(gp_done, 1)

            # ------------- scalar (ACT): W exp setup, loop exp/ln ----------
            @block.scalar
            def _(eng):
                eng.wait_ge(gp_done, 1)
                for t in range(NBLK):
                    eng.wait_ge(su_dma[t % 2], 16 * (t // 2 + 1))
                    eng.activation(
                        wt[:, t * P : (t + 1) * P], tmp[t % 2][:, :], AF.Exp,
                        bias=ln2n_t[:, :],
                    ).then_inc(su_exp, 1)
                eng.wait_ge(su_misc, 48)
                if pe_only:
                    eng.activation(wbuf[1][:, :], v[:, :], AF.Exp)
                # w(1) = exp(v0) into wbuf[0]
                eng.activation(wbuf[0][:, :], v[:, :], AF.Exp).then_inc(
                    w_sem, 1
                )
                if pe_only:
                    eng.br(block.end_bb)
                    return
                r_es = eng.alloc_register("r_es")
                r_pe1 = eng.alloc_register("r_pe1")
                r_i = eng.alloc_register("r_i")
                eng.reg_mov(r_es, 0)
                eng.reg_mov(r_pe1, 0)
                eng.reg_mov(r_i, 0)
                eng.br("act_loop")
                with nc.body("act_loop"):
                    for u in range(UNROLL):
                        eng.reg_add(r_es, r_es, 1)
                        eng.reg_add(r_pe1, r_pe1, 1)
                        eng.wait_ge(es_sem, r_es)     # es(s) written
                        eng.activation(
                            tes[u][:, :], esb[u][:, :], AF.Exp
                        ).then_inc(act_tes, 1)
                        eng.wait_ge(pe1, r_pe1)       # psum_m ready
                        eng.activation(
                            m_sb[:, :], psum_m[:, :], AF.Ln
                        ).then_inc(act_m, 1)
                    eng.reg_add(r_i, r_i, 1)
                    eng.br_lt(r_i, n_iter, "act_loop", "act_fin")
                with nc.body("act_fin"):
                    eng.br(block.end_bb)

            # ------------- tensor (PE): bcast + matvec + skinny -------------
            @block.tensor
            def _(eng):
                r_w = eng.alloc_register("r_w")
                r_prev = eng.alloc_register("r_prev")  # s-1 targets
                r_i = eng.alloc_register("r_i")
                eng.reg_mov(r_w, 0)
                eng.reg_mov(r_prev, 0)
                eng.reg_mov(r_i, 0)
                eng.wait_ge(su_misc, 48)
                eng.br("pe_loop")
                with nc.body("pe_loop"):
                    for u in range(UNROLL):
                        eng.reg_add(r_w, r_w, 1)
                        eng.wait_ge(w_sem, r_w)       # w(s) + psum_mv free
                        if not pe_only:
                            eng.wait_ge(act_m, r_prev)   # m_sb(s-1), psum_m free
                            eng.wait_ge(es_sem, r_prev)  # psum_b free
                            eng.matmul(
                                psum_b[:, :],
                                ones_row[:, :],
                                m_sb[:, :],
                                start=True,
                                stop=True,
                            ).then_inc(pe2, 1)
                        for g in range(NGRP):
                            for k in range(NSLOT):
                                t = k * NGRP + g
                                mm = eng.matmul(
                                    psum_mv[:, g : g + 1],
                                    wt[:, t * P : (t + 1) * P],
                                    wbuf[u][:, k : k + 1],
                                    start=(k == 0),
                                    stop=(k == NSLOT - 1),
                                )
                                if g == NGRP - 1 and k == NSLOT - 1:
                                    mm.then_inc(pe_mv, 1)
                        for k in range(NSLOT):
                            mm = eng.matmul(
                                psum_m[:, :],
                                colsum_bf[:, k : k + 1],
                                wbuf[u][:, k : k + 1],
                                start=(k == 0),
                                stop=(k == NSLOT - 1),
                            )
                            if k == NSLOT - 1:
                                mm.then_inc(pe1, 1)
                        eng.reg_add(r_prev, r_prev, 1)
                        if pe_only:
                            # free-running: reuse wbuf[0] every step
                            eng.reg_mov(r_w, 0)
                    eng.reg_add(r_i, r_i, 1)
                    eng.br_lt(r_i, n_iter, "pe_loop", "pe_fin")
                with nc.body("pe_fin"):
                    eng.br(block.end_bb)

            # ------------- vector (DVE): colsum setup + per-step ops -------
            @block.vector
            def _(eng):
                for k in range(NSLOT):
                    eng.wait_ge(su_exp, (k + 1) * NGRP)
                    eng.tensor_reduce(
                        colsum[:, k : k + 1],
                        wt[:, k * NGRP * P : (k + 1) * NGRP * P],
                        axis=AX.X,
                        op=ALU.add,
                    )
                eng.drain()
                eng.tensor_copy(colsum_bf[:, :], colsum[:, :]).then_inc(
                    su_misc, 16
                )
                if pe_only:
                    eng.br(block.end_bb)
                    return
                r_pe2 = eng.alloc_register("r_pe2")
                r_mv = eng.alloc_register("r_mv")
                r_tes = eng.alloc_register("r_tes")
                r_h = eng.alloc_register("r_h")
                r_i = eng.alloc_register("r_i")
                eng.reg_mov(r_pe2, 0)
                eng.reg_mov(r_mv, 0)
                eng.reg_mov(r_tes, 0)
                eng.reg_mov(r_h, 0)
                eng.reg_mov(r_i, 0)

                def dve_body(u):
                    # step s: u = (s-1) % 2; reads wbuf-free parity 1-u
                    eng.reg_add(r_pe2, r_pe2, 1)
                    eng.reg_add(r_mv, r_mv, 1)
                    eng.reg_add(r_tes, r_tes, 1)
                    if u == 0:
                        eng.reg_add(r_h, r_h, 16)
                    eng.wait_ge(h_ready[u % 2], r_h)
                    eng.wait_ge(pe2, r_pe2)       # psum_b = mh(s-1) bcast
                    eng.tensor_scalar(
                        esb[u][:, :],
                        h_step[u][:, :],
                        psum_b[:, :],
                        None,
                        op0=ALU.subtract,
                    )
                    eng.tensor_tensor(
                        c_acc[:, :], c_acc[:, :], psum_b[0:1, 0:1],
                        op=ALU.add,
                    ).then_inc(es_sem, 1)
                    eng.wait_ge(pe_mv, r_mv)      # matvec(s) done
                    eng.wait_ge(act_tes, r_tes)   # t_es(s) ready
                    eng.tensor_tensor(
                        wbuf[1 - u][:, :], psum_mv[:, :], tes[u][:, :],
                        op=ALU.mult,
                    ).then_inc(w_sem, 1)

                eng.br("dve_loop1")
                with nc.body("dve_loop1"):
                    for u in range(UNROLL):
                        dve_body(u)
                    eng.reg_add(r_i, r_i, 1)
                    eng.br_lt(r_i, snap_iter, "dve_loop1", "dve_snap")
                with nc.body("dve_snap"):
                    # state after step W0 (even): w(W0+1) sits in wbuf[1-u]
                    # with u = (W0-1) % 2 = 1 -> wbuf[0]
                    eng.drain()
                    eng.tensor_copy(w_snap[:, :], wbuf[(snap_step) % 2][:, :])
                    eng.tensor_copy(c_snap[:, :], c_acc[:, :]).then_inc(
                        snap_sem, 1
                    )
                    eng.br("dve_loop2")
                with nc.body("dve_loop2"):
                    for u in range(UNROLL):
                        dve_body(u)
                    eng.reg_add(r_i, r_i, 1)
                    eng.br_lt(r_i, n_iter, "dve_loop2", "dve_fin")
                with nc.body("dve_fin"):
                    eng.drain()
                    eng.tensor_copy(w_fin[:, :], wbuf[n_steps % 2][:, :])
                    eng.drain()
                    eng.nop().then_inc(fin_sem, 1)
                    eng.br(block.end_bb)

    nc.compile()
    return nc


_NC_CACHE = {}


def _get_nc(n_steps=NSTEP):
    if n_steps not in _NC_CACHE:
        _NC_CACHE[n_steps] = build_kernel(n_steps)
    return _NC_CACHE[n_steps]


def prep_in_maps(h, transitions):
    h = np.ascontiguousarray(np.asarray(h, dtype=np.float32))
    tr = np.ascontiguousarray(np.asarray(transitions, dtype=np.float32))
    # p-major tag layout: tag j <-> (p = j // NSLOT, k = j % NSLOT)
    wtb = np.empty((NBLK, P, P), dtype=np.float32)
    for k in range(NSLOT):
        for g in range(NGRP):
            wtb[k * NGRP + g] = tr[g::NGRP, :][:, k::NSLOT].T
    wtb = np.ascontiguousarray(wtb)
    in_maps = []
    for c in range(NCORE):
        lo = 0 if c == 0 else K * c
        hs = np.ascontiguousarray(h[lo : lo + NSTEP])
        assert hs.shape[0] == NSTEP
        if c == 0:
            v0 = np.full((T,), -10000.0, dtype=np.float32)
            v0[0] = 0.0
        else:
            v0 = np.zeros((T,), dtype=np.float32)
        in_maps.append(
            {
                "wtb": wtb,
                "hsb": hs,
                "v0f": np.ascontiguousarray(v0.reshape(P, NSLOT)),
            }
        )
    return in_maps


def _lse(x):
    m = x.max()
    return float(m + np.log(np.exp(x - m).sum()))


def _logw(wvec):
    out = np.full(wvec.shape, -1e4, dtype=np.float64)
    pos = wvec > 0
    out[pos] = np.log(wvec[pos])
    return out


def stitch(results, transitions):
    tr_end = np.asarray(transitions, dtype=np.float64)[1]  # END_IDX = 1
    vf = [_logw(np.asarray(r["wfin"], np.float64).reshape(T)) for r in results]
    cf = [float(np.asarray(r["cfin"]).reshape(-1)[0]) for r in results]
    vs = [_logw(np.asarray(r["wsnp"], np.float64).reshape(T)) for r in results]
    cs = [float(np.asarray(r["csnp"]).reshape(-1)[0]) for r in results]
    total = cf[0] + _lse(vf[0])
    for c in range(1, NCORE):
        total += (cf[c] + _lse(vf[c])) - (cs[c] + _lse(vs[c]))
    vhat = vf[NCORE - 1] - _lse(vf[NCORE - 1])
    total += _lse(vhat + tr_end)
    total += S * LN2  # undo the exp(tr - ln2) scaling
    return np.float32(total)


def kernel(h, transitions):
    from concourse.bass_utils import run_bass_kernel_spmd

    in_maps = prep_in_maps(h, transitions)
    nc = _get_nc()
    res = run_bass_kernel_spmd(nc, in_maps, list(range(NCORE)))
    return stitch(res.results, transitions)


if __name__ == "__main__":
    import reference

    inputs = {k: np.asarray(v) for k, v in reference.setup_inputs().items()}
    out = kernel(**inputs)
    print("kernel out:", out)


# revision 54
# speedup vs baseline: 1.1150x; 1.1150x over previous
"""Trainium2 Bass kernel for the CRF forward algorithm (nn_CRF).

Reference computes: scan over S=8192 steps of
    fv'[i] = logsumexp_j(fv[j] + transitions[i, j]) + h[s, i]
then logsumexp(fv + transitions[END_IDX]).

Algorithm (two levels):

1. Exp-space LINEAR scan with a lagged single-element normalizer.
   W = exp(tr - ln2) (bf16, on device; the /2 scale is undone exactly on
   the host with +8192*ln2).  State is carried LINEARLY as w:
    per step s:
        mv      = W @ w_s                      (PE matvec, fp32 psum)
        mh_s    = ln(mv[2])                    (ACT Ln of one psum elem)
        es_s    = h[s] - mh_{s-1}              (DVE, off critical path)
        t_es    = exp(es_s)                    (ACT, off critical path)
        w_{s+1} = mv * t_es                    (DVE, 8 column-pair mults)
        C      += mh_{s-1}
   Identity: exp(fv_s) = w_{s+1} * e^{C_s}, so the log-state is never
   needed on device -- the host takes ln() of the dumped w.  ANY mh
   sequence keeps fv = ln(w)+C exact; mh only controls w's dynamic
   range, and the lagged single-element normalizer keeps ln(w) in
   ~[-15, +16] for these inputs (measured) -- bf16-safe.
   The matvec runs column-chain-major (for g: for k:) so PSUM
   accumulation chains are sequential (start=True clears the whole
   bank's has_written bits -- interleaved chains are illegal).  DVE
   multiplies column PAIRS as their chains complete (pe_mv fires after
   each odd column), so by the time step s+1 needs w slot k (first at
   tile k+1 of column 0), the pair mult has already run: the only
   boundary stall is the last pair's mult vs the first ~15 matmuls.
   w is double-buffered by step parity.

2. Sequence parallelism via filter forgetting: the CRF forward filter
   forgets its initial condition in <16 steps (measured: log-direction
   error ~1e-15 after 16 steps -- dense random transitions mix fast).
   Split S=8192 into 8 chunks with boundaries e_c = W0 + K*(c+1),
   K=(S-W0)/8.  Core 0 runs rows [0, e_0) from the true init (exact).
   Core c>=1 runs rows [e_{c-1}-W0, e_c) from a uniform init: after the
   W0-step warmup its state direction equals the true filter state at
   e_{c-1}; a snapshot (w_snap, C_snap) is taken there.  Chunk log-gain
   G_c = (C_fin + lse(ln w_fin)) - (C_snap + lse(ln w_snap)) is exact
   given the snapshot direction.  Host stitches in float64:
   ans = (C_fin_0 + lse_0) + sum_c G_c
         + lse(lnw_7 - lse_7 + transitions[END_IDX]) + 8192*ln2.
   Zero inter-core communication; each core does W0+K = 1038 steps
   instead of 8192.

Layout: tag j -> (partition p = j//16, slot k = j%16); w/h tiles are
[128, 16].  W^T lives in SBUF as 256 bf16 tiles [128 j, 128 i], tile
t = k*16 + g at free offset t*128 (k = j-slot, g = i-group).

(Cross-core remote-DMA crashes this runtime and in-loop collectives have
a ~5us floor -- both dead ends; the warmup scheme needs neither.)
"""
import sys

sys.path.insert(0, "/opt/trn_rl_repo")

import numpy as np

S = 8192
T = 2048
P = 128
NSLOT = T // P          # 16 j-slots
NGRP = T // P           # 16 i-groups
NBLK = NSLOT * NGRP     # 256 W tiles
UNROLL = 2              # steps per loop iteration (buffer parity)
LN2 = 0.6931471805599453
W0 = 16                 # warmup steps (forgetting window)
K = (S - W0) // 8       # chunk length = 1022
NSTEP = W0 + K          # per-core steps = 1038
NCORE = 8
NPAIR = NSLOT // 2      # 8 column pairs


def build_kernel(n_steps=NSTEP, snap_step=W0, timing_mode=False,
                 pe_only=False, wt_fp8=False):
    import concourse.bacc as bacc
    import concourse.bass as bass
    import concourse.mybir as mybir
    from contextlib import ExitStack

    assert n_steps % UNROLL == 0 and snap_step % UNROLL == 0
    assert 0 < snap_step < n_steps
    assert not pe_only or timing_mode
    fp32 = mybir.dt.float32
    bf16 = mybir.dt.bfloat16
    fp8 = mybir.dt.float8e4
    AF = mybir.ActivationFunctionType
    ALU = mybir.AluOpType
    AX = mybir.AxisListType

    nc = bacc.Bacc("TRN2", target_bir_lowering=True, num_devices=8)

    n_wtb = 2 if timing_mode else NBLK
    n_hsb = 2 if timing_mode else n_steps
    wtb = nc.declare_dram_parameter("wtb", [n_wtb, P, P], fp32, isOutput=False)
    hsb = nc.declare_dram_parameter("hsb", [n_hsb, T], fp32, isOutput=False)
    v0f = nc.declare_dram_parameter("v0f", [P, NSLOT], fp32, isOutput=False)
    wfin_d = nc.declare_dram_parameter("wfin", [P, NSLOT], fp32, isOutput=True)
    cfin_d = nc.declare_dram_parameter("cfin", [1, 1], fp32, isOutput=True)
    wsnp_d = nc.declare_dram_parameter("wsnp", [P, NSLOT], fp32, isOutput=True)
    csnp_d = nc.declare_dram_parameter("csnp", [1, 1], fp32, isOutput=True)

    ctx = ExitStack()
    sb = lambda name, shape, dt: ctx.enter_context(nc.sbuf_tensor(name, shape, dt))
    ps = lambda name, shape, dt: ctx.enter_context(nc.psum_tensor(name, shape, dt))
    sem = lambda name: ctx.enter_context(nc.semaphore(name))

    with ctx:
        wt = sb("wt", [P, NBLK * P], fp8 if wt_fp8 else bf16)
        v = sb("v", [P, NSLOT], fp32)        # init log-state (v0 load)
        wbuf = [sb(f"wbuf{i}", [P, NSLOT], bf16) for i in range(2)]
        tes = [sb(f"tes{i}", [P, NSLOT], fp32) for i in range(2)]
        esb = [sb(f"esb{i}", [P, NSLOT], fp32) for i in range(2)]
        h_step = [sb(f"h_step{i}", [P, NSLOT], fp32) for i in range(UNROLL)]
        tmp = [sb(f"tmp{i}", [P, P], fp32) for i in range(2)]
        ln2n_t = sb("ln2n_t", [P, 1], fp32)
        ones_row = sb("ones_row", [1, P], fp32)
        m_sb = sb("m_sb", [1, 1], fp32)      # mh scalar (lagged)
        c_acc = sb("c_acc", [1, 1], fp32)    # C accumulator
        w_snap = sb("w_snap", [P, NSLOT], fp32)
        c_snap = sb("c_snap", [1, 1], fp32)
        w_fin = sb("w_fin", [P, NSLOT], fp32)

        # PSUM: one tensor per bank.  A bank may only be READ once all its
        # accumulation chains are closed, so columns are spread across
        # banks: cols (2b, 2b+1) -> psum_c[b] for b<6, cols 12-15 ->
        # psum_c[6].  With column-chain-major MM order, bank b's chains
        # close when col 2b+1 (resp. 15) finishes; DVE multiplies that
        # bank while PE accumulates into later banks.
        psum_c = [ps(f"psum_c{b}", [P, 2], fp32) for b in range(6)]
        psum_c.append(ps("psum_c6", [P, 4], fp32))
        psum_b = ps("psum_b", [P, 1], fp32)

        def mv_col(g):
            if g < 12:
                return psum_c[g // 2][:, g % 2 : g % 2 + 1]
            return psum_c[6][:, g - 12 : g - 11]

        su_dma = [sem("su_dma0"), sem("su_dma1")]  # wtb DMAs by parity
        su_exp = sem("su_exp")       # setup exp done (+1 per block)
        su_misc = sem("su_misc")     # consts / v0 ready
        h_ready = [sem("h_ready0"), sem("h_ready1")]  # h DMA by parity
        w_sem = sem("w_sem")         # w group ready (+7 init; +1 per group)
        pe_mv = sem("pe_mv")         # psum bank done (+7 per step)
        NMUL = 7                     # 6 pair mults + 1 quad mult per step
        pe2 = sem("pe2")             # PE mh-bcast done (+1 per step)
        act_tes = sem("act_tes")     # ACT exp(es) done (+1 per step)
        act_m = sem("act_m")         # ACT Ln->m_sb done (+1 per step)
        es_sem = sem("es_sem")       # DVE es + c_acc done (+1 per step)
        snap_sem = sem("snap_sem")   # snapshot copies done
        fin_sem = sem("fin_sem")     # final copies done
        gp_done = sem("gp_done")     # gpsimd consts ready

        n_iter = n_steps // UNROLL
        snap_iter = snap_step // UNROLL

        with nc.Block() as block:

            # ---------------- sync engine: all input DMAs ----------------
            @block.sync
            def _(eng):
                eng.dma_start(v[:, :], v0f[:, :]).then_inc(su_misc, 16)
                for t in range(NBLK):
                    if t >= 2:
                        eng.wait_ge(su_exp, t - 1)
                    eng.dma_start(
                        tmp[t % 2][:, :],
                        wtb[(t % 2 if timing_mode else t), :, :],
                    ).then_inc(su_dma[t % 2], 16)
                for s in range(UNROLL):
                    eng.dma_start(
                        h_step[s][:, :], hsb[s : s + 1, :]
                    ).then_inc(h_ready[s % 2], 16)
                r_off = eng.alloc_register("r_off")   # step index
                r_g = eng.alloc_register("r_g")       # pacing guard
                r_i = eng.alloc_register("r_i")
                eng.reg_mov(r_off, UNROLL)
                eng.reg_mov(r_g, 0)
                eng.reg_mov(r_i, 0)
                eng.br("sync_loop")
                with nc.body("sync_loop"):
                    for u in range(UNROLL):
                        if pe_only:
                            eng.reg_add(r_g, r_g, 7)
                            eng.wait_ge(pe_mv, r_g)
                        else:
                            eng.reg_add(r_g, r_g, 1)
                            eng.wait_ge(es_sem, r_g)
                        eng.dma_start(
                            h_step[u][:, :],
                            hsb[u : u + 1, :]
                            if timing_mode
                            else hsb[bass.ds(eng.snap(r_off), 1), :],
                        ).then_inc(h_ready[u % 2], 16)
                        if not timing_mode:
                            eng.reg_add(r_off, r_off, 1)
                    eng.reg_add(r_i, r_i, 1)
                    eng.br_lt(r_i, n_iter - 1, "sync_loop", "sync_done")
                with nc.body("sync_done"):
                    if pe_only:
                        eng.wait_ge(pe_mv, 7 * n_steps)
                    else:
                        eng.wait_ge(snap_sem, 1)
                        eng.wait_ge(fin_sem, 1)
                    eng.dma_start(wfin_d[:, :], w_fin[:, :]).then_inc(su_misc, 16)
                    eng.dma_start(cfin_d[:, :], c_acc[:, :]).then_inc(su_misc, 16)
                    eng.dma_start(wsnp_d[:, :], w_snap[:, :]).then_inc(su_misc, 16)
                    eng.dma_start(csnp_d[:, :], c_snap[:, :]).then_inc(su_misc, 16)
                    eng.wait_ge(su_misc, 96)
                    eng.br(block.end_bb)

            # ---------------- gpsimd: constants only ----------------
            @block.gpsimd
            def _(eng):
                eng.memset(ln2n_t[:, :], -LN2)
                eng.memset(ones_row[:, :], 1.0)
                eng.memset(c_acc[:, :], 0.0)
                eng.memset(m_sb[:, :], 0.0)
                eng.drain()
                eng.nop().then_inc(su_misc, 16)
                eng.nop().then_inc(gp_done, 1)

            # ------------- scalar (ACT): W exp setup, loop exp/ln ----------
            @block.scalar
            def _(eng):
                eng.wait_ge(gp_done, 1)
                for t in range(NBLK):
                    eng.wait_ge(su_dma[t % 2], 16 * (t // 2 + 1))
                    eng.activation(
                        wt[:, t * P : (t + 1) * P], tmp[t % 2][:, :], AF.Exp,
                        bias=ln2n_t[:, :],
                    ).then_inc(su_exp, 1)
                eng.wait_ge(su_misc, 32)
                if pe_only:
                    eng.activation(wbuf[1][:, :], v[:, :], AF.Exp)
                # w(1) = exp(v0) into wbuf[0]; all 7 groups at once
                eng.activation(wbuf[0][:, :], v[:, :], AF.Exp).then_inc(
                    w_sem, 7
                )
                if pe_only:
                    eng.br(block.end_bb)
                    return
                r_es = eng.alloc_register("r_es")
                r_pm = eng.alloc_register("r_pm")   # pe_mv target for Ln
                r_i = eng.alloc_register("r_i")
                eng.reg_mov(r_es, 0)
                eng.reg_mov(r_pm, 0)
                eng.reg_mov(r_i, 0)
                eng.br("act_loop")
                with nc.body("act_loop"):
                    for u in range(UNROLL):
                        eng.reg_add(r_es, r_es, 1)
                        eng.reg_add(r_pm, r_pm, 2)   # bank 1 (cols 2,3) done
                        eng.wait_ge(es_sem, r_es)    # es(s) written
                        eng.activation(
                            tes[u][:, :], esb[u][:, :], AF.Exp
                        ).then_inc(act_tes, 1)
                        eng.wait_ge(pe_mv, r_pm)     # psum col 2 complete
                        eng.activation(
                            m_sb[:, :], psum_c[1][0:1, 0:1], AF.Ln
                        ).then_inc(act_m, 1)
                        eng.reg_add(r_pm, r_pm, 5)   # rest of the step
                    eng.reg_add(r_i, r_i, 1)
                    eng.br_lt(r_i, n_iter, "act_loop", "act_fin")
                with nc.body("act_fin"):
                    eng.br(block.end_bb)

            # ------- tensor (PE): bcast + column-chain matvec --------------
            @block.tensor
            def _(eng):
                r_wq = eng.alloc_register("r_wq")
                r_prev = eng.alloc_register("r_prev")  # s-1 targets
                r_i = eng.alloc_register("r_i")
                eng.reg_mov(r_wq, 0)
                eng.reg_mov(r_prev, 0)
                eng.reg_mov(r_i, 0)
                eng.wait_ge(su_misc, 32)
                if pe_only:
                    eng.wait_ge(w_sem, 8)
                eng.br("pe_loop")
                with nc.body("pe_loop"):
                    for u in range(UNROLL):
                        if not pe_only:
                            eng.wait_ge(act_m, r_prev)   # m_sb(s-1) ready
                            eng.wait_ge(es_sem, r_prev)  # psum_b free
                            eng.matmul(
                                psum_b[:, :],
                                ones_row[:, :],
                                m_sb[:, :],
                                start=True,
                                stop=True,
                            ).then_inc(pe2, 1)
                        for g in range(NGRP):
                            for k in range(NSLOT):
                                if (g == 0 and k % 2 == 0 and k < 12
                                        and not pe_only):
                                    # first use of w slot group k//2
                                    eng.reg_add(r_wq, r_wq, 1)
                                    eng.wait_ge(w_sem, r_wq)
                                if g == 0 and k == 12 and not pe_only:
                                    # quad group (slots 12-15)
                                    eng.reg_add(r_wq, r_wq, 1)
                                    eng.wait_ge(w_sem, r_wq)
                                t = k * NGRP + g
                                mm = eng.matmul(
                                    mv_col(g),
                                    wt[:, t * P : (t + 1) * P],
                                    wbuf[0 if pe_only else u][:, k : k + 1],
                                    start=(k == 0),
                                    stop=(k == NSLOT - 1),
                                )
                                if k == NSLOT - 1 and (
                                    (g % 2 == 1 and g < 12) or g == 15
                                ):
                                    mm.then_inc(pe_mv, 1)  # bank done
                        eng.reg_add(r_prev, r_prev, 1)
                    eng.reg_add(r_i, r_i, 1)
                    eng.br_lt(r_i, n_iter, "pe_loop", "pe_fin")
                with nc.body("pe_fin"):
                    eng.br(block.end_bb)

            # ------------- vector (DVE): per-step es + pair mults ----------
            @block.vector
            def _(eng):
                if pe_only:
                    eng.br(block.end_bb)
                    return
                r_pe2 = eng.alloc_register("r_pe2")
                r_mv = eng.alloc_register("r_mv")
                r_tes = eng.alloc_register("r_tes")
                r_m = eng.alloc_register("r_m")
                r_h = eng.alloc_register("r_h")
                r_i = eng.alloc_register("r_i")
                eng.reg_mov(r_pe2, 0)
                eng.reg_mov(r_mv, 0)
                eng.reg_mov(r_tes, 0)
                eng.reg_mov(r_m, 0)
                eng.reg_mov(r_h, 0)
                eng.reg_mov(r_i, 0)
                eng.wait_ge(su_misc, 32)

                def dve_body(u):
                    # step s: u = (s-1) % 2
                    eng.reg_add(r_pe2, r_pe2, 1)
                    eng.reg_add(r_tes, r_tes, 1)
                    eng.reg_add(r_m, r_m, 1)
                    if u == 0:
                        eng.reg_add(r_h, r_h, 16)
                    eng.wait_ge(h_ready[u % 2], r_h)
                    eng.wait_ge(pe2, r_pe2)       # psum_b = mh(s-1) bcast
                    eng.tensor_scalar(
                        esb[u][:, :],
                        h_step[u][:, :],
                        psum_b[:, :],
                        None,
                        op0=ALU.subtract,
                    )
                    eng.tensor_tensor(
                        c_acc[:, :], c_acc[:, :], psum_b[0:1, 0:1],
                        op=ALU.add,
                    ).then_inc(es_sem, 1)
                    for b in range(7):
                        lo = 2 * b
                        wd = 2 if b < 6 else 4
                        eng.reg_add(r_mv, r_mv, 1)
                        eng.wait_ge(pe_mv, r_mv)  # psum bank b closed
                        if b == 0:
                            eng.wait_ge(act_tes, r_tes)  # t_es(s) ready
                        if b == 1:
                            eng.wait_ge(act_m, r_m)      # Ln(s) read psum c2
                        eng.tensor_tensor(
                            wbuf[1 - u][:, lo : lo + wd],
                            psum_c[b][:, 0:wd],
                            tes[u][:, lo : lo + wd],
                            op=ALU.mult,
                        ).then_inc(w_sem, 1)

                eng.br("dve_loop1")
                with nc.body("dve_loop1"):
                    for u in range(UNROLL):
                        dve_body(u)
                    eng.reg_add(r_i, r_i, 1)
                    eng.br_lt(r_i, snap_iter, "dve_loop1", "dve_snap")
                with nc.body("dve_snap"):
                    # w(W0+1) sits in wbuf[W0 % 2]
                    eng.drain()
                    eng.tensor_copy(w_snap[:, :], wbuf[snap_step % 2][:, :])
                    eng.tensor_copy(c_snap[:, :], c_acc[:, :]).then_inc(
                        snap_sem, 1
                    )
                    eng.br("dve_loop2")
                with nc.body("dve_loop2"):
                    for u in range(UNROLL):
                        dve_body(u)
                    eng.reg_add(r_i, r_i, 1)
                    eng.br_lt(r_i, n_iter, "dve_loop2", "dve_fin")
                with nc.body("dve_fin"):
                    eng.drain()
                    eng.tensor_copy(w_fin[:, :], wbuf[n_steps % 2][:, :])
                    eng.drain()
                    eng.nop().then_inc(fin_sem, 1)
                    eng.br(block.end_bb)

    nc.compile()
    return nc


_NC_CACHE = {}


def _get_nc(n_steps=NSTEP):
    if n_steps not in _NC_CACHE:
        _NC_CACHE[n_steps] = build_kernel(n_steps)
    return _NC_CACHE[n_steps]


def prep_in_maps(h, transitions):
    h = np.ascontiguousarray(np.asarray(h, dtype=np.float32))
    tr = np.ascontiguousarray(np.asarray(transitions, dtype=np.float32))
    # p-major tag layout: tag j <-> (p = j // NSLOT, k = j % NSLOT)
    wtb = np.empty((NBLK, P, P), dtype=np.float32)
    for k in range(NSLOT):
        for g in range(NGRP):
            wtb[k * NGRP + g] = tr[g::NGRP, :][:, k::NSLOT].T
    wtb = np.ascontiguousarray(wtb)
    in_maps = []
    for c in range(NCORE):
        lo = 0 if c == 0 else K * c
        hs = np.ascontiguousarray(h[lo : lo + NSTEP])
        assert hs.shape[0] == NSTEP
        if c == 0:
            v0 = np.full((T,), -10000.0, dtype=np.float32)
            v0[0] = 0.0
        else:
            v0 = np.zeros((T,), dtype=np.float32)
        in_maps.append(
            {
                "wtb": wtb,
                "hsb": hs,
                "v0f": np.ascontiguousarray(v0.reshape(P, NSLOT)),
            }
        )
    return in_maps


def _lse(x):
    m = x.max()
    return float(m + np.log(np.exp(x - m).sum()))


def _logw(wvec):
    out = np.full(wvec.shape, -1e4, dtype=np.float64)
    pos = wvec > 0
    out[pos] = np.log(wvec[pos])
    return out


def stitch(results, transitions):
    tr_end = np.asarray(transitions, dtype=np.float64)[1]  # END_IDX = 1
    vf = [_logw(np.asarray(r["wfin"], np.float64).reshape(T)) for r in results]
    cf = [float(np.asarray(r["cfin"]).reshape(-1)[0]) for r in results]
    vs = [_logw(np.asarray(r["wsnp"], np.float64).reshape(T)) for r in results]
    cs = [float(np.asarray(r["csnp"]).reshape(-1)[0]) for r in results]
    total = cf[0] + _lse(vf[0])
    for c in range(1, NCORE):
        total += (cf[c] + _lse(vf[c])) - (cs[c] + _lse(vs[c]))
    vhat = vf[NCORE - 1] - _lse(vf[NCORE - 1])
    total += _lse(vhat + tr_end)
    total += S * LN2  # undo the exp(tr - ln2) scaling
    return np.float32(total)


def kernel(h, transitions):
    from concourse.bass_utils import run_bass_kernel_spmd

    in_maps = prep_in_maps(h, transitions)
    nc = _get_nc()
    res = run_bass_kernel_spmd(nc, in_maps, list(range(NCORE)))
    return stitch(res.results, transitions)


if __name__ == "__main__":
    import reference

    inputs = {k: np.asarray(v) for k, v in reference.setup_inputs().items()}
    out = kernel(**inputs)
    print("kernel out:", out)
